# revision 17
# baseline (speedup 1.0000x reference)
"""Trainium2 Bass kernel for nn_Deformation (NeRF-style deformation field).

Pure data-parallel over the batch dim: 2048 batch rows -> 8 cores x 256 rows.
Each core processes 32768 points through:
  fourier embedding (8 active bands, sin/cos via range-reduced ACT Sin)
  + latent-code projection broadcast, 3-layer ReLU MLP, tiny head -> screw axis s[6]
  + closed-form se(3) exponential applied to x, plus scalar reduction partials.

Layouts (per core):
  point index pt = b_local*128 + s_idx  (b_local in [0,256), s_idx in [0,128))
  chunk j = 512 points (4 batch rows); pack = 2 chunks -> [128, 512] tiles
  "planes": [128, 256] per component c: plane[s_idx, b_local] = val[pt]
Host does only sharding / replication / layout prep + final gather; all math
(normalization, projection, MLP, trig, deformation, reductions) runs on device.
"""
import numpy as np
from contextlib import ExitStack

import concourse.bass as bass
import concourse.bacc as bacc
import concourse.mybir as mybir
import concourse.tile as tile
from concourse.bass_utils import run_bass_kernel_spmd
from concourse.masks import make_identity
from concourse.tile_rust import add_dep_helper

F32 = mybir.dt.float32
A = mybir.AluOpType
AF = mybir.ActivationFunctionType

N_CORES = 8
B_TOT, S_PTS = 2048, 128
B_CORE = B_TOT // N_CORES          # 256
NPTS = B_CORE * S_PTS              # 32768
NCHUNK = NPTS // 512               # 64
NPACK = NCHUNK // 2                # 32
MAGIC = float(np.float32(1.5 * 2 ** 23))
TWO_PI = float(2.0 * np.pi)
IPE_VAR = 1e-4
N_FREQS = 16


def _build_program(b_out_host):
    """Build the per-core Bass program. b_out values are needed host-side only
    for nothing (bias is applied via a [6,1] tile), kept for signature clarity."""
    nc = bacc.Bacc("TRN2", target_bir_lowering=False, debug=False)

    d_xrep = nc.dram_tensor("xrep", (NPACK, 128, 512), F32, kind="ExternalInput")
    d_xpl = nc.dram_tensor("xpl", (128, 768), F32, kind="ExternalInput")
    d_codes = nc.dram_tensor("codes", (B_CORE, 128), F32, kind="ExternalInput")
    d_wproj = nc.dram_tensor("wproj", (128, 128), F32, kind="ExternalInput")
    d_bproj = nc.dram_tensor("bproj", (128,), F32, kind="ExternalInput")
    d_w0e = nc.dram_tensor("w0e", (128, 128), F32, kind="ExternalInput")
    d_w0dc = nc.dram_tensor("w0dc", (128, 128), F32, kind="ExternalInput")
    d_b0 = nc.dram_tensor("b0", (128,), F32, kind="ExternalInput")
    d_w1 = nc.dram_tensor("w1", (128, 128), F32, kind="ExternalInput")
    d_b1 = nc.dram_tensor("b1", (128,), F32, kind="ExternalInput")
    d_w2 = nc.dram_tensor("w2", (128, 128), F32, kind="ExternalInput")
    d_b2 = nc.dram_tensor("b2", (128,), F32, kind="ExternalInput")
    d_wout = nc.dram_tensor("wout", (128, 6), F32, kind="ExternalInput")
    d_bout = nc.dram_tensor("bout", (6,), F32, kind="ExternalInput")
    d_s128 = nc.dram_tensor("s128", (128,), F32, kind="ExternalInput")
    d_c128 = nc.dram_tensor("c128", (128,), F32, kind="ExternalInput")
    d_blk = nc.dram_tensor("blk", (72, 1024), F32, kind="ExternalInput")

    d_ypl = nc.dram_tensor("ypl", (128, 768), F32, kind="ExternalOutput")
    d_part = nc.dram_tensor("partials", (128, 4), F32, kind="ExternalOutput")

    d_sstage = nc.dram_tensor("sstage", (6, NPTS), F32)   # internal DRAM staging
    d_dcwst = nc.dram_tensor("dcwst", (2, 128, 128), F32)  # dcw scatter bounce

    with tile.TileContext(nc) as tc:
        with tc.tile_pool(name="singles", bufs=1) as sg:
            # ---- persistent weights / constants ----
            w0e = sg.tile([128, 128], F32)
            nc.sync.dma_start(out=w0e, in_=d_w0e[:, :])
            w0dc = sg.tile([128, 128], F32)
            nc.sync.dma_start(out=w0dc, in_=d_w0dc[:, :])
            w1t = sg.tile([128, 128], F32)
            nc.sync.dma_start(out=w1t, in_=d_w1[:, :])
            w2t = sg.tile([128, 128], F32)
            nc.sync.dma_start(out=w2t, in_=d_w2[:, :])
            woutt = sg.tile([128, 6], F32)
            nc.sync.dma_start(out=woutt, in_=d_wout[:, :])
            wpt = sg.tile([128, 128], F32)
            nc.sync.dma_start(out=wpt, in_=d_wproj[:, :])
            b0t = sg.tile([128, 1], F32)
            nc.sync.dma_start(out=b0t, in_=d_b0[:, None])
            b1t = sg.tile([128, 1], F32)
            nc.sync.dma_start(out=b1t, in_=d_b1[:, None])
            b2t = sg.tile([128, 1], F32)
            nc.sync.dma_start(out=b2t, in_=d_b2[:, None])
            bpt = sg.tile([128, 1], F32)
            nc.sync.dma_start(out=bpt, in_=d_bproj[:, None])
            boutt = sg.tile([6, 1], F32)
            nc.sync.dma_start(out=boutt, in_=d_bout[:, None])
            s128t = sg.tile([128, 1], F32)
            nc.sync.dma_start(out=s128t, in_=d_s128[:, None])
            c128t = sg.tile([128, 1], F32)
            nc.sync.dma_start(out=c128t, in_=d_c128[:, None])
            blkt = sg.tile([72, 1024], F32)
            nc.sync.dma_start(out=blkt, in_=d_blk[:, :])
            identt = sg.tile([128, 128], F32)
            make_identity(nc, identt)
            halfpit = sg.tile([128, 1], F32)
            nc.vector.memset(halfpit, float(np.pi / 2))
            # dcw8 halves: lhsT rows for the per-chunk latent broadcast matmul;
            # lo copy at partitions 0..7 (chunk A), hi copy at 64..71 (chunk B)
            dcw8lo = [sg.tile([8, 16 * 128], F32, name=f"dcw8lo_{h}") for h in range(2)]
            dcw8hi = [sg.tile([72, 16 * 128], F32, name=f"dcw8hi_{h}") for h in range(2)]
            xplt = sg.tile([128, 768], F32)
            nc.sync.dma_start(out=xplt, in_=d_xpl[:, :])

            # ---- front: codes -> normalize -> project -> dcw rows ----
            with tc.tile_pool(name="front", bufs=2) as fp, \
                 tc.tile_pool(name="front_ps", bufs=2, space="PSUM") as fps:
                for h in range(2):
                    ct = fp.tile([128, 128], F32, tag="codes")
                    nc.sync.dma_start(out=ct, in_=d_codes[128 * h:128 * (h + 1), :])
                    sq = fp.tile([128, 128], F32, tag="sq")
                    nc.vector.tensor_tensor(sq, ct, ct, A.mult)
                    rs = fp.tile([128, 1], F32, tag="rs")
                    nc.vector.tensor_reduce(rs, sq, mybir.AxisListType.X, A.add)
                    nrm = fp.tile([128, 1], F32, tag="nrm")
                    nc.scalar.activation(nrm, rs, AF.Sqrt)
                    inv = fp.tile([128, 1], F32, tag="inv")
                    nc.vector.reciprocal(inv, nrm)
                    dcn = fp.tile([128, 128], F32, tag="dcn")
                    nc.vector.tensor_scalar(dcn, ct, inv, None, A.mult)
                    tps = fps.tile([128, 128], F32, tag="tps")
                    nc.tensor.transpose(tps, dcn, identt)
                    dcnT = fp.tile([128, 128], F32, tag="dcnT")
                    nc.vector.tensor_copy(dcnT, tps)
                    pps = fps.tile([128, 128], F32, tag="pps")
                    nc.tensor.matmul(pps, lhsT=wpt, rhs=dcnT, start=True, stop=True)
                    projT = fp.tile([128, 128], F32, tag="projT")
                    nc.scalar.activation(projT, pps, AF.Identity, bias=bpt, scale=1.0)
                    dps = fps.tile([128, 128], F32, tag="dps")
                    nc.tensor.matmul(dps, lhsT=projT, rhs=w0dc, start=True, stop=True)
                    dst = fp.tile([128, 128], F32, tag="dst")
                    nc.vector.tensor_copy(dst, dps)
                    # scatter rows r=8*jj+i -> dcw8*[h][i, jj*128+f] via DRAM bounce
                    dw1 = nc.sync.dma_start(out=d_dcwst[h, :, :], in_=dst)
                    in_ap = d_dcwst[h, :, :].rearrange("(jj i) f -> i jj f", i=8)
                    out_ap = dcw8lo[h].rearrange("i (jj f) -> i jj f", f=128)
                    dw2 = nc.sync.dma_start(out=out_ap, in_=in_ap)
                    add_dep_helper(dw2.ins, dw1.ins, sync=True, reason="dcw bounce RAW")
                    out_hi = dcw8hi[h][64:72, :].rearrange("i (jj f) -> i jj f", f=128)
                    dw3 = nc.sync.dma_start(out=out_hi, in_=in_ap)
                    add_dep_helper(dw3.ins, dw1.ins, sync=True, reason="dcw bounce RAW hi")

            # ---- main loop over 32 packs (2 chunks each) ----
            _tail_stack = ExitStack()
            pl = _tail_stack.enter_context(tc.tile_pool(name="pl", bufs=1))
            dp = _tail_stack.enter_context(tc.tile_pool(name="dtmp", bufs=1))
            if True:
                Rpl = pl.tile([128, 768], F32, name="Rpl")
                Dpl = pl.tile([128, 768], F32, name="Dpl")
                yplt = pl.tile([128, 768], F32, name="yplt")
                part = pl.tile([128, 4], F32, name="part")
                _vt_n = [0]

                def emit_tail_half(p0, deps):
                    _vt_n[0] = 0
                    sl = slice(p0, p0 + 64)
                    for tgt, c0 in ((Rpl, 0), (Dpl, 3)):
                        out_ap = tgt[sl, :].rearrange("p (c n) -> p c n", c=3)
                        in_ap = bass.AP(
                            tensor=d_sstage[:, :].tensor,
                            offset=c0 * NPTS + p0 * 256,
                            ap=[[256, 64], [NPTS, 3], [1, 256]],
                        )
                        g = nc.sync.dma_start(out=out_ap, in_=in_ap)
                        for sw in deps:
                            add_dep_helper(g.ins, sw.ins, sync=True,
                                           reason="sstage RAW")

                    def wv(t):
                        return t[sl, :].rearrange("p (c n) -> p c n", c=3)

                    def bc(t):
                        return t[sl, None, :].to_broadcast((64, 3, 256))

                    def vtile(w=256):
                        _vt_n[0] += 1
                        return dp.tile([128, w], F32, name=f"dt{_vt_n[0]}")

                    R = [Rpl[sl, 256 * c:256 * (c + 1)] for c in range(3)]
                    D = [Dpl[sl, 256 * c:256 * (c + 1)] for c in range(3)]
                    X = [xplt[sl, 256 * c:256 * (c + 1)] for c in range(3)]

                    q = vtile(768)
                    nc.scalar.activation(wv(q), wv(Rpl), AF.Square)
                    qd = vtile(768)
                    nc.scalar.activation(wv(qd), wv(Dpl), AF.Square)
                    rxw = vtile(768)
                    nc.vector.tensor_tensor(rxw[sl, :], Rpl[sl, :], xplt[sl, :], A.mult)
                    rdw = vtile(768)
                    nc.gpsimd.tensor_tensor(rdw[sl, :], Rpl[sl, :], Dpl[sl, :], A.mult)
                    t2a, t2 = vtile(), vtile()
                    nc.vector.tensor_tensor(t2a[sl], q[sl, 0:256], q[sl, 256:512], A.add)
                    nc.vector.tensor_tensor(t2[sl], t2a[sl], q[sl, 512:768], A.add)
                    sd2a, sd2 = vtile(), vtile()
                    nc.gpsimd.tensor_tensor(sd2a[sl], qd[sl, 0:256], qd[sl, 256:512], A.add)
                    nc.gpsimd.tensor_tensor(sd2[sl], sd2a[sl], qd[sl, 512:768], A.add)
                    rdxa, rdx = vtile(), vtile()
                    nc.vector.tensor_tensor(rdxa[sl], rxw[sl, 0:256], rxw[sl, 256:512], A.add)
                    nc.vector.tensor_tensor(rdx[sl], rdxa[sl], rxw[sl, 512:768], A.add)
                    rdda, rdd = vtile(), vtile()
                    nc.gpsimd.tensor_tensor(rdda[sl], rdw[sl, 0:256], rdw[sl, 256:512], A.add)
                    nc.gpsimd.tensor_tensor(rdd[sl], rdda[sl], rdw[sl, 512:768], A.add)

                    t_ = vtile()
                    nc.scalar.activation(t_[sl], t2[sl], AF.Sqrt)
                    invt, invt2 = vtile(), vtile()
                    nc.vector.reciprocal(invt[sl], t_[sl])
                    nc.vector.reciprocal(invt2[sl], t2[sl])
                    hs, hsq = vtile(), vtile()
                    nc.scalar.activation(hs[sl], t_[sl], AF.Sin, bias=0.0, scale=0.5)
                    nc.vector.tensor_tensor(hsq[sl], hs[sl], hs[sl], A.mult)
                    ctq, vq = vtile(), vtile()
                    nc.vector.tensor_scalar(ctq[sl], hsq[sl], -2.0, 1.0, A.mult, A.add)
                    nc.vector.tensor_scalar(vq[sl], ctq[sl], -1.0, 1.0, A.mult, A.add)
                    B2 = vtile()
                    nc.vector.tensor_tensor(B2[sl], vq[sl], invt2[sl], A.mult)
                    a1, t2a2, w_, stq = vtile(), vtile(), vtile(), vtile()
                    nc.vector.tensor_scalar(a1[sl], t2[sl], -1.0 / 120.0, 1.0 / 6.0,
                                            A.mult, A.add)
                    nc.gpsimd.tensor_tensor(t2a2[sl], t2[sl], a1[sl], A.mult)
                    nc.gpsimd.tensor_tensor(w_[sl], t_[sl], t2a2[sl], A.mult)
                    nc.vector.tensor_tensor(stq[sl], t_[sl], w_[sl], A.subtract)
                    Acf = vtile()
                    nc.vector.tensor_tensor(Acf[sl], stq[sl], invt2[sl], A.mult)
                    Bq, Bi = vtile(), vtile()
                    nc.vector.tensor_tensor(Bq[sl], B2[sl], invt2[sl], A.mult)
                    nc.vector.tensor_tensor(Bi[sl], B2[sl], invt[sl], A.mult)
                    tm, i3, C3, C5 = vtile(), vtile(), vtile(), vtile()
                    nc.vector.tensor_tensor(tm[sl], t_[sl], stq[sl], A.subtract)
                    nc.gpsimd.tensor_tensor(i3[sl], invt[sl], invt2[sl], A.mult)
                    nc.vector.tensor_tensor(C3[sl], tm[sl], i3[sl], A.mult)
                    nc.vector.tensor_tensor(C5[sl], C3[sl], invt2[sl], A.mult)
                    u1, v1 = vtile(), vtile()
                    nc.scalar.activation(u1[sl], B2[sl], AF.Identity, bias=1.0, scale=-1.0)
                    nc.scalar.activation(v1[sl], C3[sl], AF.Identity, bias=1.0, scale=-1.0)
                    Wa, Wb, W12 = vtile(), vtile(), vtile()
                    nc.vector.tensor_tensor(Wa[sl], Bq[sl], rdx[sl], A.mult)
                    nc.vector.tensor_tensor(Wb[sl], C5[sl], rdd[sl], A.mult)
                    nc.vector.tensor_tensor(W12[sl], Wa[sl], Wb[sl], A.add)

                    crx = vtile(768)
                    crd = vtile(768)
                    for c in range(3):
                        c1, c2 = (c + 1) % 3, (c + 2) % 3
                        m1, m2 = vtile(), vtile()
                        nc.vector.tensor_tensor(m1[sl], R[c1], X[c2], A.mult)
                        nc.vector.tensor_tensor(m2[sl], R[c2], X[c1], A.mult)
                        nc.vector.tensor_tensor(crx[sl, 256 * c:256 * (c + 1)],
                                                m1[sl], m2[sl], A.subtract)
                    for c in range(3):
                        c1, c2 = (c + 1) % 3, (c + 2) % 3
                        m1, m2 = vtile(), vtile()
                        nc.gpsimd.tensor_tensor(m1[sl], R[c1], D[c2], A.mult)
                        nc.gpsimd.tensor_tensor(m2[sl], R[c2], D[c1], A.mult)
                        nc.gpsimd.tensor_tensor(crd[sl, 256 * c:256 * (c + 1)],
                                                m1[sl], m2[sl], A.subtract)

                    m1w, m2w, m3w, m4w, m5w = (vtile(768) for _ in range(5))
                    nc.vector.tensor_tensor(wv(m1w), wv(xplt), bc(u1), A.mult)
                    nc.vector.tensor_tensor(wv(m2w), wv(crx), bc(Acf), A.mult)
                    nc.vector.tensor_tensor(wv(m3w), wv(Rpl), bc(W12), A.mult)
                    nc.gpsimd.tensor_tensor(wv(m4w), wv(Dpl), bc(v1), A.mult)
                    nc.gpsimd.tensor_tensor(wv(m5w), wv(crd), bc(Bi), A.mult)
                    e1, e2, e3 = vtile(768), vtile(768), vtile(768)
                    nc.vector.tensor_tensor(e1[sl], m1w[sl], m2w[sl], A.add)
                    nc.gpsimd.tensor_tensor(e2[sl], m3w[sl], m4w[sl], A.add)
                    nc.vector.tensor_tensor(e3[sl], e1[sl], e2[sl], A.add)
                    nc.vector.tensor_tensor(yplt[sl, :], e3[sl], m5w[sl], A.add)
                    nc.vector.memset(part[sl, :], 0.0)
                    nc.vector.tensor_reduce(part[sl, 0:1], t2[sl],
                                            mybir.AxisListType.X, A.add)
                    nc.vector.tensor_reduce(part[sl, 1:2], sd2[sl],
                                            mybir.AxisListType.X, A.add)
                    nc.vector.tensor_reduce(part[sl, 2:3], t_[sl],
                                            mybir.AxisListType.X, A.add)

            with tc.tile_pool(name="xr", bufs=4) as xp_pool, \
                 tc.tile_pool(name="wk", bufs=3) as wk, \
                 tc.tile_pool(name="hh", bufs=4) as hh, \
                 tc.tile_pool(name="ps0", bufs=2, space="PSUM") as ps0, \
                 tc.tile_pool(name="ps1", bufs=2, space="PSUM") as ps1, \
                 tc.tile_pool(name="ps2", bufs=2, space="PSUM") as ps2, \
                 tc.tile_pool(name="pss", bufs=2, space="PSUM") as pss:
                sstage_writes = []

                def stage_sincos(pt2):
                    xr = xp_pool.tile([128, 512], F32, tag="xr")
                    nc.sync.dma_start(out=xr, in_=d_xrep[pt2, :, :])
                    tt = wk.tile([128, 512], F32, tag="tt")
                    nc.gpsimd.tensor_scalar(tt, xr, s128t, c128t, A.mult, A.add)
                    ut = wk.tile([128, 512], F32, tag="ut")
                    nc.vector.tensor_scalar(ut, tt, MAGIC, MAGIC, A.add, A.subtract)
                    vt = wk.tile([128, 512], F32, tag="vt")
                    nc.gpsimd.tensor_tensor(vt, tt, ut, A.subtract)
                    embt = wk.tile([128, 512], F32, tag="emb")
                    nc.scalar.activation(embt, vt, AF.Sin, bias=0.0, scale=TWO_PI)
                    return embt

                def stage_h0(pt2, embt):
                    h, csl = pt2 // 16, slice((pt2 % 16) * 128, (pt2 % 16 + 1) * 128)
                    h0ps = []
                    for ch in range(2):
                        p = ps0.tile([128, 512], F32, tag="h0")
                        nc.tensor.matmul(p, lhsT=w0e[64 * ch:64 * ch + 64, :],
                                         rhs=embt[64 * ch:64 * ch + 64, :],
                                         start=True, stop=False)
                        h0ps.append(p)
                    nc.tensor.matmul(h0ps[0], lhsT=dcw8lo[h][:, csl],
                                     rhs=blkt[0:8, 0:512], start=False, stop=True)
                    nc.tensor.matmul(h0ps[1], lhsT=dcw8hi[h][64:72, csl],
                                     rhs=blkt[64:72, 512:1024], start=False, stop=True)
                    return h0ps

                def stage_shead(pt2, h2):
                    p = pss.tile([38, 512], F32, tag="sps")
                    nc.tensor.matmul(p[0:6, :], lhsT=woutt, rhs=h2[0],
                                     start=True, stop=True, tile_position=(0, 0))
                    nc.tensor.matmul(p[32:38, :], lhsT=woutt, rhs=h2[1],
                                     start=True, stop=True, tile_position=(0, 32))
                    for ch in range(2):
                        j = 2 * pt2 + ch
                        s_sb = hh.tile([6, 512], F32, tag="ssb")
                        nc.vector.tensor_scalar(s_sb, p[32 * ch:32 * ch + 6, :],
                                                boutt, None, A.add)
                        sw = nc.sync.dma_start(
                            out=d_sstage[:, 512 * j:512 * (j + 1)], in_=s_sb)
                        sstage_writes.append(sw)

                def stage_mlp(pt2, h0ps):
                    h0 = []
                    for ch in range(2):
                        t_ = hh.tile([128, 512], F32, tag="h0s")
                        nc.scalar.activation(t_, h0ps[ch], AF.Relu, bias=b0t, scale=1.0)
                        h0.append(t_)
                    h1 = []
                    for ch in range(2):
                        p = ps1.tile([128, 512], F32, tag="h1")
                        nc.tensor.matmul(p, lhsT=w1t, rhs=h0[ch], start=True, stop=True)
                        t_ = hh.tile([128, 512], F32, tag="h1s")
                        nc.scalar.activation(t_, p, AF.Relu, bias=b1t, scale=1.0)
                        h1.append(t_)
                    h2 = []
                    for ch in range(2):
                        p = ps2.tile([128, 512], F32, tag="h2")
                        nc.tensor.matmul(p, lhsT=w2t, rhs=h1[ch], start=True, stop=True)
                        t_ = hh.tile([128, 512], F32, tag="h2s")
                        nc.vector.tensor_scalar(t_, p, b2t, 0.0, A.add, A.max)
                        h2.append(t_)
                    return h2

                # software-pipelined emission: sincos 2 ahead, s-head 1 behind
                emb_q = {}
                h2_q = {}
                emb_q[0] = stage_sincos(0)
                emb_q[1] = stage_sincos(1)
                for pt2 in range(NPACK):
                    if pt2 + 2 < NPACK:
                        emb_q[pt2 + 2] = stage_sincos(pt2 + 2)
                    h0ps = stage_h0(pt2, emb_q.pop(pt2))
                    if pt2 - 1 in h2_q:
                        stage_shead(pt2 - 1, h2_q.pop(pt2 - 1))
                    h2_q[pt2] = stage_mlp(pt2, h0ps)
                stage_shead(NPACK - 1, h2_q.pop(NPACK - 1))

            emit_tail_half(0, list(sstage_writes))
            emit_tail_half(64, list(sstage_writes))
            nc.sync.dma_start(out=d_ypl[:, :], in_=yplt)
            nc.sync.dma_start(out=d_part[:, :], in_=part)
            _tail_stack.close()

    nc.compile()
    return nc


def _host_inputs(x, deformation_codes, w_proj, b_proj, w0, b0, w1, b1, w2, b2,
                 w_out, b_out, decayscale):
    ds = float(np.asarray(decayscale))
    f_idx = np.arange(N_FREQS, dtype=np.float64)
    freqs = 2.0 ** f_idx
    window = 0.5 * (1.0 - np.cos(np.pi * np.clip(ds - f_idx, 0.0, 1.0)))
    atten = np.exp(-0.5 * (freqs ** 2) * IPE_VAR)
    wf = (window * atten).astype(np.float32)
    if np.any(wf[8:] != 0):
        raise NotImplementedError("kernel compiled for <=8 active fourier bands")

    w0 = np.asarray(w0, np.float32)
    # w0e64 rows (t2, f8, k4): row 32t+4f+k ~ wf[f] * w0[6f+3t+k]; k=3 zero pad
    w0e64 = np.zeros((64, 128), np.float32)
    for t in range(2):
        for f in range(8):
            for k in range(3):
                w0e64[32 * t + 4 * f + k] = wf[f] * w0[6 * f + 3 * t + k]
    w0e128 = np.vstack([w0e64, w0e64]).astype(np.float32)

    s64 = np.zeros(64, np.float32)
    c64 = np.zeros(64, np.float32)
    for t in range(2):
        for f in range(8):
            for k in range(3):
                s64[32 * t + 4 * f + k] = np.float32(2.0 ** f / (2 * np.pi))
                c64[32 * t + 4 * f + k] = 0.25 if t == 1 else 0.0
    s128 = np.concatenate([s64, s64]).astype(np.float32)
    c128 = np.concatenate([c64, c64]).astype(np.float32)

    blk = np.zeros((72, 1024), np.float32)
    for i in range(8):
        blk[i, 128 * i:128 * (i + 1)] = 1.0
        blk[64 + i, 128 * i:128 * (i + 1)] = 1.0

    common = {
        "wproj": np.asarray(w_proj, np.float32),
        "bproj": np.asarray(b_proj, np.float32),
        "w0e": w0e128,
        "w0dc": np.ascontiguousarray(w0[96:224]).astype(np.float32),
        "b0": np.asarray(b0, np.float32),
        "w1": np.asarray(w1, np.float32), "b1": np.asarray(b1, np.float32),
        "w2": np.asarray(w2, np.float32), "b2": np.asarray(b2, np.float32),
        "wout": np.asarray(w_out, np.float32),
        "bout": np.asarray(b_out, np.float32),
        "s128": s128, "c128": c128, "blk": blk,
    }

    x = np.asarray(x, np.float32)
    dc = np.asarray(deformation_codes, np.float32)
    in_maps = []
    for cidx in range(N_CORES):
        xs = x[B_CORE * cidx:B_CORE * (cidx + 1)]          # [256, 128, 3]
        xT4 = np.zeros((4, NPTS), np.float32)
        xT4[:3] = xs.reshape(NPTS, 3).T
        base = xT4.reshape(4, NCHUNK, 512)                  # [k, j, l]
        bjl = base.transpose(1, 0, 2)                       # [j, k, l]
        rep = np.broadcast_to(bjl[:, None, None, :, :], (NCHUNK, 2, 8, 4, 512))
        xrep = np.ascontiguousarray(rep).reshape(NPACK, 2, 64, 512).reshape(NPACK, 128, 512)
        xpl = np.ascontiguousarray(
            xs.reshape(NPTS, 3).reshape(128, 256, 3).transpose(0, 2, 1)).reshape(128, 768)
        m = dict(common)
        m["xrep"] = np.ascontiguousarray(xrep)
        m["xpl"] = xpl
        m["codes"] = np.ascontiguousarray(dc[B_CORE * cidx:B_CORE * (cidx + 1)])
        in_maps.append(m)
    return in_maps


def _assemble(results):
    shards = []
    p0 = p1 = p2 = 0.0
    for res in results:
        ypl = res["ypl"]                                    # [128, 768]
        shards.append(
            ypl.reshape(128, 3, 256).transpose(0, 2, 1).reshape(B_CORE, S_PTS, 3))
        part = res["partials"].astype(np.float64)
        p0 += part[:, 0].sum()
        p1 += part[:, 1].sum()
        p2 += part[:, 2].sum()
    xt = np.concatenate(shards, axis=0).astype(np.float32)  # [2048, 128, 3]
    n = float(B_TOT * S_PTS)
    loss = np.float32((p0 + p1) / (n * 6.0))
    rot = np.float32(p2 / n * (180.0 / np.pi))
    trans = np.float32(p1 / (n * 3.0))
    return xt, loss, rot, trans


_NC_CACHE = {}


def _get_program():
    if "nc" not in _NC_CACHE:
        _NC_CACHE["nc"] = _build_program(None)
    return _NC_CACHE["nc"]


def kernel(x, deformation_codes, w_proj, b_proj, w0, b0, w1, b1, w2, b2,
           w_out, b_out, decayscale, _run_kwargs=None):
    nc = _get_program()
    in_maps = _host_inputs(x, deformation_codes, w_proj, b_proj, w0, b0,
                           w1, b1, w2, b2, w_out, b_out, decayscale)
    kw = _run_kwargs or {}
    res = run_bass_kernel_spmd(nc, in_maps, core_ids=list(range(N_CORES)), **kw)
    out = _assemble(res.results)
    if _run_kwargs is not None:
        return out, res
    return out


# revision 18
# speedup vs baseline: 1.1083x; 1.1083x over previous
"""Trainium2 Bass kernel for nn_Deformation (NeRF-style deformation field).

Pure data-parallel over the batch dim: 2048 batch rows -> 8 cores x 256 rows.
Each core processes 32768 points through:
  fourier embedding (8 active bands, sin/cos via range-reduced ACT Sin)
  + latent-code projection broadcast, 3-layer ReLU MLP, tiny head -> screw axis s[6]
  + closed-form se(3) exponential applied to x, plus scalar reduction partials.

Layouts (per core):
  point index pt = b_local*128 + s_idx  (b_local in [0,256), s_idx in [0,128))
  chunk j = 512 points (4 batch rows); pack = 2 chunks -> [128, 512] tiles
  "planes": [128, 256] per component c: plane[s_idx, b_local] = val[pt]
Host does only sharding / replication / layout prep + final gather; all math
(normalization, projection, MLP, trig, deformation, reductions) runs on device.
"""
import numpy as np
from contextlib import ExitStack

import concourse.bass as bass
import concourse.bacc as bacc
import concourse.mybir as mybir
import concourse.tile as tile
from concourse.bass_utils import run_bass_kernel_spmd
from concourse.masks import make_identity
from concourse.tile_rust import add_dep_helper

F32 = mybir.dt.float32
A = mybir.AluOpType
AF = mybir.ActivationFunctionType

N_CORES = 8
B_TOT, S_PTS = 2048, 128
B_CORE = B_TOT // N_CORES          # 256
NPTS = B_CORE * S_PTS              # 32768
NCHUNK = NPTS // 512               # 64
NPACK = NCHUNK // 2                # 32
MAGIC = float(np.float32(1.5 * 2 ** 23))
TWO_PI = float(2.0 * np.pi)
IPE_VAR = 1e-4
N_FREQS = 16


def _build_program(b_out_host):
    """Build the per-core Bass program. b_out values are needed host-side only
    for nothing (bias is applied via a [6,1] tile), kept for signature clarity."""
    nc = bacc.Bacc("TRN2", target_bir_lowering=False, debug=False)

    d_xrep = nc.dram_tensor("xrep", (NPACK, 128, 512), F32, kind="ExternalInput")
    d_xpl = nc.dram_tensor("xpl", (128, 768), F32, kind="ExternalInput")
    d_codes = nc.dram_tensor("codes", (B_CORE, 128), F32, kind="ExternalInput")
    d_wproj = nc.dram_tensor("wproj", (128, 128), F32, kind="ExternalInput")
    d_bproj = nc.dram_tensor("bproj", (128,), F32, kind="ExternalInput")
    d_w0e = nc.dram_tensor("w0e", (128, 128), F32, kind="ExternalInput")
    d_w0dc = nc.dram_tensor("w0dc", (128, 128), F32, kind="ExternalInput")
    d_b0 = nc.dram_tensor("b0", (128,), F32, kind="ExternalInput")
    d_w1 = nc.dram_tensor("w1", (128, 128), F32, kind="ExternalInput")
    d_b1 = nc.dram_tensor("b1", (128,), F32, kind="ExternalInput")
    d_w2 = nc.dram_tensor("w2", (128, 128), F32, kind="ExternalInput")
    d_b2 = nc.dram_tensor("b2", (128,), F32, kind="ExternalInput")
    d_wout = nc.dram_tensor("wout", (128, 6), F32, kind="ExternalInput")
    d_bout = nc.dram_tensor("bout", (6,), F32, kind="ExternalInput")
    d_s128 = nc.dram_tensor("s128", (128,), F32, kind="ExternalInput")
    d_c128 = nc.dram_tensor("c128", (128,), F32, kind="ExternalInput")
    d_blk = nc.dram_tensor("blk", (72, 1024), F32, kind="ExternalInput")

    d_ypl = nc.dram_tensor("ypl", (128, 768), F32, kind="ExternalOutput")
    d_part = nc.dram_tensor("partials", (128, 4), F32, kind="ExternalOutput")

    d_sstage = nc.dram_tensor("sstage", (6, NPTS), F32)   # internal DRAM staging
    d_dcwst = nc.dram_tensor("dcwst", (2, 128, 128), F32)  # dcw scatter bounce

    with tile.TileContext(nc) as tc:
        with tc.tile_pool(name="singles", bufs=1) as sg:
            # ---- persistent weights / constants ----
            w0e = sg.tile([128, 128], F32)
            nc.sync.dma_start(out=w0e, in_=d_w0e[:, :])
            w0dc = sg.tile([128, 128], F32)
            nc.sync.dma_start(out=w0dc, in_=d_w0dc[:, :])
            w1t = sg.tile([128, 128], F32)
            nc.sync.dma_start(out=w1t, in_=d_w1[:, :])
            w2t = sg.tile([128, 128], F32)
            nc.sync.dma_start(out=w2t, in_=d_w2[:, :])
            woutt = sg.tile([128, 6], F32)
            nc.sync.dma_start(out=woutt, in_=d_wout[:, :])
            wpt = sg.tile([128, 128], F32)
            nc.sync.dma_start(out=wpt, in_=d_wproj[:, :])
            b0t = sg.tile([128, 1], F32)
            nc.sync.dma_start(out=b0t, in_=d_b0[:, None])
            b1t = sg.tile([128, 1], F32)
            nc.sync.dma_start(out=b1t, in_=d_b1[:, None])
            b2t = sg.tile([128, 1], F32)
            nc.sync.dma_start(out=b2t, in_=d_b2[:, None])
            bpt = sg.tile([128, 1], F32)
            nc.sync.dma_start(out=bpt, in_=d_bproj[:, None])
            boutt = sg.tile([6, 1], F32)
            nc.sync.dma_start(out=boutt, in_=d_bout[:, None])
            s128t = sg.tile([128, 1], F32)
            nc.sync.dma_start(out=s128t, in_=d_s128[:, None])
            c128t = sg.tile([128, 1], F32)
            nc.sync.dma_start(out=c128t, in_=d_c128[:, None])
            blkt = sg.tile([72, 1024], F32)
            nc.sync.dma_start(out=blkt, in_=d_blk[:, :])
            identt = sg.tile([128, 128], F32)
            make_identity(nc, identt)
            halfpit = sg.tile([128, 1], F32)
            nc.vector.memset(halfpit, float(np.pi / 2))
            # dcw8 halves: lhsT rows for the per-chunk latent broadcast matmul;
            # lo copy at partitions 0..7 (chunk A), hi copy at 64..71 (chunk B)
            dcw8lo = [sg.tile([8, 16 * 128], F32, name=f"dcw8lo_{h}") for h in range(2)]
            dcw8hi = [sg.tile([72, 16 * 128], F32, name=f"dcw8hi_{h}") for h in range(2)]
            xplt = sg.tile([128, 768], F32)
            nc.sync.dma_start(out=xplt, in_=d_xpl[:, :])

            # ---- front: codes -> normalize -> project -> dcw rows ----
            with tc.tile_pool(name="front", bufs=2) as fp, \
                 tc.tile_pool(name="front_ps", bufs=2, space="PSUM") as fps:
                for h in range(2):
                    ct = fp.tile([128, 128], F32, tag="codes")
                    nc.sync.dma_start(out=ct, in_=d_codes[128 * h:128 * (h + 1), :])
                    sq = fp.tile([128, 128], F32, tag="sq")
                    nc.vector.tensor_tensor(sq, ct, ct, A.mult)
                    rs = fp.tile([128, 1], F32, tag="rs")
                    nc.vector.tensor_reduce(rs, sq, mybir.AxisListType.X, A.add)
                    nrm = fp.tile([128, 1], F32, tag="nrm")
                    nc.scalar.activation(nrm, rs, AF.Sqrt)
                    inv = fp.tile([128, 1], F32, tag="inv")
                    nc.vector.reciprocal(inv, nrm)
                    dcn = fp.tile([128, 128], F32, tag="dcn")
                    nc.vector.tensor_scalar(dcn, ct, inv, None, A.mult)
                    tps = fps.tile([128, 128], F32, tag="tps")
                    nc.tensor.transpose(tps, dcn, identt)
                    dcnT = fp.tile([128, 128], F32, tag="dcnT")
                    nc.vector.tensor_copy(dcnT, tps)
                    pps = fps.tile([128, 128], F32, tag="pps")
                    nc.tensor.matmul(pps, lhsT=wpt, rhs=dcnT, start=True, stop=True)
                    projT = fp.tile([128, 128], F32, tag="projT")
                    nc.scalar.activation(projT, pps, AF.Identity, bias=bpt, scale=1.0)
                    dps = fps.tile([128, 128], F32, tag="dps")
                    nc.tensor.matmul(dps, lhsT=projT, rhs=w0dc, start=True, stop=True)
                    dst = fp.tile([128, 128], F32, tag="dst")
                    nc.vector.tensor_copy(dst, dps)
                    # scatter rows r=8*jj+i -> dcw8*[h][i, jj*128+f] via DRAM bounce
                    dw1 = nc.sync.dma_start(out=d_dcwst[h, :, :], in_=dst)
                    in_ap = d_dcwst[h, :, :].rearrange("(jj i) f -> i jj f", i=8)
                    out_ap = dcw8lo[h].rearrange("i (jj f) -> i jj f", f=128)
                    dw2 = nc.sync.dma_start(out=out_ap, in_=in_ap)
                    add_dep_helper(dw2.ins, dw1.ins, sync=True, reason="dcw bounce RAW")
                    out_hi = dcw8hi[h][64:72, :].rearrange("i (jj f) -> i jj f", f=128)
                    dw3 = nc.sync.dma_start(out=out_hi, in_=in_ap)
                    add_dep_helper(dw3.ins, dw1.ins, sync=True, reason="dcw bounce RAW hi")

            # ---- main loop over 32 packs (2 chunks each) ----
            _tail_stack = ExitStack()
            pl = _tail_stack.enter_context(tc.tile_pool(name="pl", bufs=1))
            dp = _tail_stack.enter_context(tc.tile_pool(name="dtmp", bufs=1))
            if True:
                Rpl = pl.tile([128, 768], F32, name="Rpl")
                Dpl = pl.tile([128, 768], F32, name="Dpl")
                yplt = pl.tile([128, 768], F32, name="yplt")
                part = pl.tile([128, 4], F32, name="part")
                _vt_n = [0]

                def emit_tail_half(p0, deps, n=64):
                    _vt_n[0] = 0
                    sl = slice(p0, p0 + n)
                    for tgt, c0 in ((Rpl, 0), (Dpl, 3)):
                        out_ap = tgt[sl, :].rearrange("p (c n) -> p c n", c=3)
                        in_ap = bass.AP(
                            tensor=d_sstage[:, :].tensor,
                            offset=c0 * NPTS + p0 * 256,
                            ap=[[256, n], [NPTS, 3], [1, 256]],
                        )
                        g = nc.sync.dma_start(out=out_ap, in_=in_ap)
                        for sw in deps:
                            add_dep_helper(g.ins, sw.ins, sync=True,
                                           reason="sstage RAW")

                    def wv(t):
                        return t[sl, :].rearrange("p (c n) -> p c n", c=3)

                    def bc(t):
                        return t[sl, None, :].to_broadcast((n, 3, 256))

                    def vtile(w=256):
                        _vt_n[0] += 1
                        return dp.tile([128, w], F32, name=f"dt{_vt_n[0]}")

                    R = [Rpl[sl, 256 * c:256 * (c + 1)] for c in range(3)]
                    D = [Dpl[sl, 256 * c:256 * (c + 1)] for c in range(3)]
                    X = [xplt[sl, 256 * c:256 * (c + 1)] for c in range(3)]

                    q = vtile(768)
                    nc.scalar.activation(wv(q), wv(Rpl), AF.Square)
                    qd = vtile(768)
                    nc.scalar.activation(wv(qd), wv(Dpl), AF.Square)
                    rxw = vtile(768)
                    nc.vector.tensor_tensor(rxw[sl, :], Rpl[sl, :], xplt[sl, :], A.mult)
                    rdw = vtile(768)
                    nc.gpsimd.tensor_tensor(rdw[sl, :], Rpl[sl, :], Dpl[sl, :], A.mult)
                    t2a, t2 = vtile(), vtile()
                    nc.vector.tensor_tensor(t2a[sl], q[sl, 0:256], q[sl, 256:512], A.add)
                    nc.vector.tensor_tensor(t2[sl], t2a[sl], q[sl, 512:768], A.add)
                    sd2a, sd2 = vtile(), vtile()
                    nc.gpsimd.tensor_tensor(sd2a[sl], qd[sl, 0:256], qd[sl, 256:512], A.add)
                    nc.gpsimd.tensor_tensor(sd2[sl], sd2a[sl], qd[sl, 512:768], A.add)
                    rdxa, rdx = vtile(), vtile()
                    nc.vector.tensor_tensor(rdxa[sl], rxw[sl, 0:256], rxw[sl, 256:512], A.add)
                    nc.vector.tensor_tensor(rdx[sl], rdxa[sl], rxw[sl, 512:768], A.add)
                    rdda, rdd = vtile(), vtile()
                    nc.gpsimd.tensor_tensor(rdda[sl], rdw[sl, 0:256], rdw[sl, 256:512], A.add)
                    nc.gpsimd.tensor_tensor(rdd[sl], rdda[sl], rdw[sl, 512:768], A.add)

                    t_ = vtile()
                    nc.scalar.activation(t_[sl], t2[sl], AF.Sqrt)
                    invt, invt2 = vtile(), vtile()
                    nc.vector.reciprocal(invt[sl], t_[sl])
                    nc.vector.reciprocal(invt2[sl], t2[sl])
                    hs, hsq = vtile(), vtile()
                    nc.scalar.activation(hs[sl], t_[sl], AF.Sin, bias=0.0, scale=0.5)
                    nc.vector.tensor_tensor(hsq[sl], hs[sl], hs[sl], A.mult)
                    ctq, vq = vtile(), vtile()
                    nc.vector.tensor_scalar(ctq[sl], hsq[sl], -2.0, 1.0, A.mult, A.add)
                    nc.vector.tensor_scalar(vq[sl], ctq[sl], -1.0, 1.0, A.mult, A.add)
                    B2 = vtile()
                    nc.vector.tensor_tensor(B2[sl], vq[sl], invt2[sl], A.mult)
                    a1, t2a2, w_, stq = vtile(), vtile(), vtile(), vtile()
                    nc.vector.tensor_scalar(a1[sl], t2[sl], -1.0 / 120.0, 1.0 / 6.0,
                                            A.mult, A.add)
                    nc.gpsimd.tensor_tensor(t2a2[sl], t2[sl], a1[sl], A.mult)
                    nc.gpsimd.tensor_tensor(w_[sl], t_[sl], t2a2[sl], A.mult)
                    nc.vector.tensor_tensor(stq[sl], t_[sl], w_[sl], A.subtract)
                    Acf = vtile()
                    nc.vector.tensor_tensor(Acf[sl], stq[sl], invt2[sl], A.mult)
                    Bq, Bi = vtile(), vtile()
                    nc.vector.tensor_tensor(Bq[sl], B2[sl], invt2[sl], A.mult)
                    nc.vector.tensor_tensor(Bi[sl], B2[sl], invt[sl], A.mult)
                    tm, i3, C3, C5 = vtile(), vtile(), vtile(), vtile()
                    nc.vector.tensor_tensor(tm[sl], t_[sl], stq[sl], A.subtract)
                    nc.gpsimd.tensor_tensor(i3[sl], invt[sl], invt2[sl], A.mult)
                    nc.vector.tensor_tensor(C3[sl], tm[sl], i3[sl], A.mult)
                    nc.vector.tensor_tensor(C5[sl], C3[sl], invt2[sl], A.mult)
                    u1, v1 = vtile(), vtile()
                    nc.scalar.activation(u1[sl], B2[sl], AF.Identity, bias=1.0, scale=-1.0)
                    nc.scalar.activation(v1[sl], C3[sl], AF.Identity, bias=1.0, scale=-1.0)
                    Wa, Wb, W12 = vtile(), vtile(), vtile()
                    nc.vector.tensor_tensor(Wa[sl], Bq[sl], rdx[sl], A.mult)
                    nc.vector.tensor_tensor(Wb[sl], C5[sl], rdd[sl], A.mult)
                    nc.vector.tensor_tensor(W12[sl], Wa[sl], Wb[sl], A.add)

                    crx = vtile(768)
                    crd = vtile(768)
                    for c in range(3):
                        c1, c2 = (c + 1) % 3, (c + 2) % 3
                        m1, m2 = vtile(), vtile()
                        nc.vector.tensor_tensor(m1[sl], R[c1], X[c2], A.mult)
                        nc.vector.tensor_tensor(m2[sl], R[c2], X[c1], A.mult)
                        nc.vector.tensor_tensor(crx[sl, 256 * c:256 * (c + 1)],
                                                m1[sl], m2[sl], A.subtract)
                    for c in range(3):
                        c1, c2 = (c + 1) % 3, (c + 2) % 3
                        m1, m2 = vtile(), vtile()
                        nc.gpsimd.tensor_tensor(m1[sl], R[c1], D[c2], A.mult)
                        nc.gpsimd.tensor_tensor(m2[sl], R[c2], D[c1], A.mult)
                        nc.gpsimd.tensor_tensor(crd[sl, 256 * c:256 * (c + 1)],
                                                m1[sl], m2[sl], A.subtract)

                    m1w, m2w, m3w, m4w, m5w = (vtile(768) for _ in range(5))
                    nc.vector.tensor_tensor(wv(m1w), wv(xplt), bc(u1), A.mult)
                    nc.vector.tensor_tensor(wv(m2w), wv(crx), bc(Acf), A.mult)
                    nc.vector.tensor_tensor(wv(m3w), wv(Rpl), bc(W12), A.mult)
                    nc.gpsimd.tensor_tensor(wv(m4w), wv(Dpl), bc(v1), A.mult)
                    nc.gpsimd.tensor_tensor(wv(m5w), wv(crd), bc(Bi), A.mult)
                    e1, e2, e3 = vtile(768), vtile(768), vtile(768)
                    nc.vector.tensor_tensor(e1[sl], m1w[sl], m2w[sl], A.add)
                    nc.gpsimd.tensor_tensor(e2[sl], m3w[sl], m4w[sl], A.add)
                    nc.vector.tensor_tensor(e3[sl], e1[sl], e2[sl], A.add)
                    nc.vector.tensor_tensor(yplt[sl, :], e3[sl], m5w[sl], A.add)
                    nc.vector.memset(part[sl, :], 0.0)
                    nc.vector.tensor_reduce(part[sl, 0:1], t2[sl],
                                            mybir.AxisListType.X, A.add)
                    nc.vector.tensor_reduce(part[sl, 1:2], sd2[sl],
                                            mybir.AxisListType.X, A.add)
                    nc.vector.tensor_reduce(part[sl, 2:3], t_[sl],
                                            mybir.AxisListType.X, A.add)

            with tc.tile_pool(name="xr", bufs=4) as xp_pool, \
                 tc.tile_pool(name="wk", bufs=3) as wk, \
                 tc.tile_pool(name="hh", bufs=4) as hh, \
                 tc.tile_pool(name="ps0", bufs=2, space="PSUM") as ps0, \
                 tc.tile_pool(name="ps1", bufs=2, space="PSUM") as ps1, \
                 tc.tile_pool(name="ps2", bufs=2, space="PSUM") as ps2, \
                 tc.tile_pool(name="pss", bufs=2, space="PSUM") as pss:
                sstage_writes = []

                def stage_sincos(pt2):
                    xr = xp_pool.tile([128, 512], F32, tag="xr")
                    nc.sync.dma_start(out=xr, in_=d_xrep[pt2, :, :])
                    tt = wk.tile([128, 512], F32, tag="tt")
                    nc.gpsimd.tensor_scalar(tt, xr, s128t, c128t, A.mult, A.add)
                    ut = wk.tile([128, 512], F32, tag="ut")
                    nc.vector.tensor_scalar(ut, tt, MAGIC, MAGIC, A.add, A.subtract)
                    vt = wk.tile([128, 512], F32, tag="vt")
                    nc.gpsimd.tensor_tensor(vt, tt, ut, A.subtract)
                    embt = wk.tile([128, 512], F32, tag="emb")
                    nc.scalar.activation(embt, vt, AF.Sin, bias=0.0, scale=TWO_PI)
                    return embt

                def stage_h0(pt2, embt):
                    h, csl = pt2 // 16, slice((pt2 % 16) * 128, (pt2 % 16 + 1) * 128)
                    h0ps = []
                    for ch in range(2):
                        p = ps0.tile([128, 512], F32, tag="h0")
                        nc.tensor.matmul(p, lhsT=w0e[64 * ch:64 * ch + 64, :],
                                         rhs=embt[64 * ch:64 * ch + 64, :],
                                         start=True, stop=False)
                        h0ps.append(p)
                    nc.tensor.matmul(h0ps[0], lhsT=dcw8lo[h][:, csl],
                                     rhs=blkt[0:8, 0:512], start=False, stop=True)
                    nc.tensor.matmul(h0ps[1], lhsT=dcw8hi[h][64:72, csl],
                                     rhs=blkt[64:72, 512:1024], start=False, stop=True)
                    return h0ps

                def stage_shead(pt2, h2):
                    p = pss.tile([38, 512], F32, tag="sps")
                    nc.tensor.matmul(p[0:6, :], lhsT=woutt, rhs=h2[0],
                                     start=True, stop=True, tile_position=(0, 0))
                    nc.tensor.matmul(p[32:38, :], lhsT=woutt, rhs=h2[1],
                                     start=True, stop=True, tile_position=(0, 32))
                    for ch in range(2):
                        j = 2 * pt2 + ch
                        s_sb = hh.tile([6, 512], F32, tag="ssb")
                        nc.vector.tensor_scalar(s_sb, p[32 * ch:32 * ch + 6, :],
                                                boutt, None, A.add)
                        sw = nc.sync.dma_start(
                            out=d_sstage[:, 512 * j:512 * (j + 1)], in_=s_sb)
                        sstage_writes.append(sw)

                def stage_mlp(pt2, h0ps):
                    h0 = []
                    for ch in range(2):
                        t_ = hh.tile([128, 512], F32, tag="h0s")
                        nc.scalar.activation(t_, h0ps[ch], AF.Relu, bias=b0t, scale=1.0)
                        h0.append(t_)
                    h1 = []
                    for ch in range(2):
                        p = ps1.tile([128, 512], F32, tag="h1")
                        nc.tensor.matmul(p, lhsT=w1t, rhs=h0[ch], start=True, stop=True)
                        t_ = hh.tile([128, 512], F32, tag="h1s")
                        nc.scalar.activation(t_, p, AF.Relu, bias=b1t, scale=1.0)
                        h1.append(t_)
                    h2 = []
                    for ch in range(2):
                        p = ps2.tile([128, 512], F32, tag="h2")
                        nc.tensor.matmul(p, lhsT=w2t, rhs=h1[ch], start=True, stop=True)
                        t_ = hh.tile([128, 512], F32, tag="h2s")
                        nc.vector.tensor_scalar(t_, p, b2t, 0.0, A.add, A.max)
                        h2.append(t_)
                    return h2

                # software-pipelined emission: sincos 2 ahead, s-head 1 behind
                emb_q = {}
                h2_q = {}
                emb_q[0] = stage_sincos(0)
                emb_q[1] = stage_sincos(1)
                for pt2 in range(NPACK):
                    if pt2 + 2 < NPACK:
                        emb_q[pt2 + 2] = stage_sincos(pt2 + 2)
                    h0ps = stage_h0(pt2, emb_q.pop(pt2))
                    if pt2 - 1 in h2_q:
                        stage_shead(pt2 - 1, h2_q.pop(pt2 - 1))
                    h2_q[pt2] = stage_mlp(pt2, h0ps)
                stage_shead(NPACK - 1, h2_q.pop(NPACK - 1))

            emit_tail_half(0, list(sstage_writes), n=128)
            nc.sync.dma_start(out=d_ypl[:, :], in_=yplt)
            nc.sync.dma_start(out=d_part[:, :], in_=part)
            _tail_stack.close()

    nc.compile()
    return nc


def _host_inputs(x, deformation_codes, w_proj, b_proj, w0, b0, w1, b1, w2, b2,
                 w_out, b_out, decayscale):
    ds = float(np.asarray(decayscale))
    f_idx = np.arange(N_FREQS, dtype=np.float64)
    freqs = 2.0 ** f_idx
    window = 0.5 * (1.0 - np.cos(np.pi * np.clip(ds - f_idx, 0.0, 1.0)))
    atten = np.exp(-0.5 * (freqs ** 2) * IPE_VAR)
    wf = (window * atten).astype(np.float32)
    if np.any(wf[8:] != 0):
        raise NotImplementedError("kernel compiled for <=8 active fourier bands")

    w0 = np.asarray(w0, np.float32)
    # w0e64 rows (t2, f8, k4): row 32t+4f+k ~ wf[f] * w0[6f+3t+k]; k=3 zero pad
    w0e64 = np.zeros((64, 128), np.float32)
    for t in range(2):
        for f in range(8):
            for k in range(3):
                w0e64[32 * t + 4 * f + k] = wf[f] * w0[6 * f + 3 * t + k]
    w0e128 = np.vstack([w0e64, w0e64]).astype(np.float32)

    s64 = np.zeros(64, np.float32)
    c64 = np.zeros(64, np.float32)
    for t in range(2):
        for f in range(8):
            for k in range(3):
                s64[32 * t + 4 * f + k] = np.float32(2.0 ** f / (2 * np.pi))
                c64[32 * t + 4 * f + k] = 0.25 if t == 1 else 0.0
    s128 = np.concatenate([s64, s64]).astype(np.float32)
    c128 = np.concatenate([c64, c64]).astype(np.float32)

    blk = np.zeros((72, 1024), np.float32)
    for i in range(8):
        blk[i, 128 * i:128 * (i + 1)] = 1.0
        blk[64 + i, 128 * i:128 * (i + 1)] = 1.0

    common = {
        "wproj": np.asarray(w_proj, np.float32),
        "bproj": np.asarray(b_proj, np.float32),
        "w0e": w0e128,
        "w0dc": np.ascontiguousarray(w0[96:224]).astype(np.float32),
        "b0": np.asarray(b0, np.float32),
        "w1": np.asarray(w1, np.float32), "b1": np.asarray(b1, np.float32),
        "w2": np.asarray(w2, np.float32), "b2": np.asarray(b2, np.float32),
        "wout": np.asarray(w_out, np.float32),
        "bout": np.asarray(b_out, np.float32),
        "s128": s128, "c128": c128, "blk": blk,
    }

    x = np.asarray(x, np.float32)
    dc = np.asarray(deformation_codes, np.float32)
    in_maps = []
    for cidx in range(N_CORES):
        xs = x[B_CORE * cidx:B_CORE * (cidx + 1)]          # [256, 128, 3]
        xT4 = np.zeros((4, NPTS), np.float32)
        xT4[:3] = xs.reshape(NPTS, 3).T
        base = xT4.reshape(4, NCHUNK, 512)                  # [k, j, l]
        bjl = base.transpose(1, 0, 2)                       # [j, k, l]
        rep = np.broadcast_to(bjl[:, None, None, :, :], (NCHUNK, 2, 8, 4, 512))
        xrep = np.ascontiguousarray(rep).reshape(NPACK, 2, 64, 512).reshape(NPACK, 128, 512)
        xpl = np.ascontiguousarray(
            xs.reshape(NPTS, 3).reshape(128, 256, 3).transpose(0, 2, 1)).reshape(128, 768)
        m = dict(common)
        m["xrep"] = np.ascontiguousarray(xrep)
        m["xpl"] = xpl
        m["codes"] = np.ascontiguousarray(dc[B_CORE * cidx:B_CORE * (cidx + 1)])
        in_maps.append(m)
    return in_maps


def _assemble(results):
    shards = []
    p0 = p1 = p2 = 0.0
    for res in results:
        ypl = res["ypl"]                                    # [128, 768]
        shards.append(
            ypl.reshape(128, 3, 256).transpose(0, 2, 1).reshape(B_CORE, S_PTS, 3))
        part = res["partials"].astype(np.float64)
        p0 += part[:, 0].sum()
        p1 += part[:, 1].sum()
        p2 += part[:, 2].sum()
    xt = np.concatenate(shards, axis=0).astype(np.float32)  # [2048, 128, 3]
    n = float(B_TOT * S_PTS)
    loss = np.float32((p0 + p1) / (n * 6.0))
    rot = np.float32(p2 / n * (180.0 / np.pi))
    trans = np.float32(p1 / (n * 3.0))
    return xt, loss, rot, trans


_NC_CACHE = {}


def _get_program():
    if "nc" not in _NC_CACHE:
        _NC_CACHE["nc"] = _build_program(None)
    return _NC_CACHE["nc"]


def kernel(x, deformation_codes, w_proj, b_proj, w0, b0, w1, b1, w2, b2,
           w_out, b_out, decayscale, _run_kwargs=None):
    nc = _get_program()
    in_maps = _host_inputs(x, deformation_codes, w_proj, b_proj, w0, b0,
                           w1, b1, w2, b2, w_out, b_out, decayscale)
    kw = _run_kwargs or {}
    res = run_bass_kernel_spmd(nc, in_maps, core_ids=list(range(N_CORES)), **kw)
    out = _assemble(res.results)
    if _run_kwargs is not None:
        return out, res
    return out
